# revision 1
# baseline (speedup 1.0000x reference)
"""Self-contained Trainium2 Bass kernel for 3D-RoPE multi-head attention.

Problem: x[2,2048,1020] -> qkv proj (17 heads x 60) -> 3D rotary on q,k ->
softmax attention -> out proj + bias.

Strategy: sequence-parallel across 8 NeuronCores (2 batch groups x 4 ranks,
512 rows each). Each core projects its own rows, RoPEs q/k locally, then
AllGathers rotated K^T and V (with a fused ones-column for the softmax
denominator) within its 4-core group, computes attention for its local
queries against the full 2048-key sequence, and projects the output rows.
Matmuls run in bf16 (f32 PSUM accumulation); softmax skips max-subtraction
(logits are ~N(0,1), exp is safe in f32).
"""

import sys

if "/opt/trn_rl_repo" not in sys.path:
    sys.path.insert(0, "/opt/trn_rl_repo")

import numpy as np
import ml_dtypes

HEADS = 17
DH = 60
D3 = 20
MIN_FREQ = 1.0 / 64.0
B, N, DIM = 2, 2048, 1020
NL = 512          # local rows per core
DIMP = 1024       # padded contraction dim (8 k-tiles)
SLOT = 64         # padded per-head column slot
NSLOT = 18        # 17 heads + 1 pad slot
MQK = NSLOT * SLOT  # 1152
VX = HEADS * (DH + 1)  # 1037: v columns with ones column per head
NPAIR = 9         # head pairs (last pair has only head A)
RG = [[0, 1, 2, 3], [4, 5, 6, 7]]

_nc_cache = {}


def _build_nc(dbg=None):
    from concourse import bass, tile, bacc
    import concourse.mybir as mybir
    from concourse.masks import make_identity

    BF = mybir.dt.bfloat16
    F32 = mybir.dt.float32
    AF = mybir.ActivationFunctionType
    ALU = mybir.AluOpType

    nc = bacc.Bacc("TRN2", target_bir_lowering=False, debug=False, num_devices=8)

    x_ext = nc.declare_dram_parameter("x", [NL, DIM], BF, isOutput=False)
    wqk_ext = nc.declare_dram_parameter("wqk", [2 * NPAIR, DIMP, 128], BF, isOutput=False)
    wv_ext = nc.declare_dram_parameter("wv", [DIMP, DIM], BF, isOutput=False)
    wout_ext = nc.declare_dram_parameter("wout", [MQK, DIM], BF, isOutput=False)
    cos_ext = nc.declare_dram_parameter("cos_t", [128, NL], BF, isOutput=False)
    sin_ext = nc.declare_dram_parameter("sin_t", [128, NL], BF, isOutput=False)
    perm_ext = nc.declare_dram_parameter("perm", [128, 128], BF, isOutput=False)
    out_ext = nc.declare_dram_parameter("out", [NL, DIM], F32, isOutput=True)
    dbg_ext = None
    if dbg is not None:
        dbg_shapes = {
            "xT": [DIMP, NL], "rot": [2 * MQK, NL],
            "aoT": [MQK, NL],
            "dots0": [16 * 128, 1024], "av0": [256, NL],
        }
        dbg_ext = nc.declare_dram_parameter("dbg", dbg_shapes[dbg], F32, isOutput=True)

    KT = 8           # contraction k-tiles (1024/128)
    SCALE = float(DH) ** -0.5
    # AllGather chunking: pairs/heads per chunk
    PAIRS_OF = [[0], [1, 2], [3, 4, 5], [6, 7, 8]]
    FIRST_PAIR = [0, 1, 3, 6]
    FIRST_HEAD = [0, 2, 6, 12]
    NHEADS_OF = [2, 4, 6, 5]
    CHUNK_OF_PAIR = [0, 1, 1, 2, 2, 2, 3, 3, 3]

    with tile.TileContext(nc) as tc:
        with (
            tc.tile_pool(name="per", bufs=1) as per,
            tc.tile_pool(name="wrk", bufs=2) as wrk,
            tc.tile_pool(name="expp", bufs=4) as expp,
            tc.tile_pool(name="dram", bufs=1, space="DRAM") as dram,
        ):
            # ---------- persistent SBUF loads (x + tables first) ----------
            cos_sb = per.tile([128, NL], BF, name="cos", tag="cos")
            nc.sync.dma_start(out=cos_sb[:], in_=cos_ext[:])
            sin_sb = per.tile([128, NL], BF, name="sin", tag="sin")
            nc.sync.dma_start(out=sin_sb[:], in_=sin_ext[:])
            perm_sb = per.tile([128, 128], BF, name="perm", tag="perm")
            nc.sync.dma_start(out=perm_sb[:], in_=perm_ext[:])

            # per-chunk DRAM bounce buffers for k and v collectives
            kb_loc, kb_gat, vb_loc, vb_gat = [], [], [], []
            for j in range(4):
                rows = len(PAIRS_OF[j]) * 128
                vcols = NHEADS_OF[j] * 61
                kb_loc.append(dram.tile([rows, NL], BF, name=f"kbl{j}", tag=f"kbl{j}"))
                kb_gat.append(dram.tile([4 * rows, NL], BF, name=f"kbg{j}", tag=f"kbg{j}"))
                vb_loc.append(dram.tile([NL, vcols], BF, name=f"vbl{j}", tag=f"vbl{j}"))
                vb_gat.append(dram.tile([4 * NL, vcols], BF, name=f"vbg{j}", tag=f"vbg{j}"))

            # ---------- phase 1: x -> xT (bf16) ----------
            ident = per.tile([128, 128], BF, name="ident", tag="ident")
            make_identity(nc, ident[:])
            xT_sb = []
            for k in range(KT):
                t = per.tile([128, NL], BF, name=f"xT{k}", tag=f"xT{k}")
                xT_sb.append(t)
            nc.vector.memset(xT_sb[KT - 1][:], 0.0)

            with tc.tile_pool(name="psP", bufs=2, space="PSUM") as psP:
                for mt in range(4):
                    xt = wrk.tile([128, DIM], BF, name="xrow", tag="xrow")
                    nc.sync.dma_start(out=xt[:], in_=x_ext[mt * 128:(mt + 1) * 128, :])
                    for k in range(KT):
                        kk = min(128, DIM - k * 128)  # 124 on last tile
                        pt = psP.tile([128, 128], BF, name="tp", tag="tp")
                        nc.tensor.transpose(
                            pt[0:kk, :], xt[:, k * 128:k * 128 + kk], ident[:]
                        )
                        nc.vector.tensor_copy(
                            xT_sb[k][0:kk, mt * 128:(mt + 1) * 128], pt[0:kk, :]
                        )


                # weight loads (issued on idle scalar/vector queues), v first
                wv_sb = []
                for k in range(KT):
                    t = per.tile([128, DIM], BF, name=f"wv{k}", tag=f"wv{k}")
                    nc.scalar.dma_start(out=t[:], in_=wv_ext[k * 128:(k + 1) * 128, :])
                    wv_sb.append(t)
                wqkm = [None] * (2 * NPAIR)
                for m in list(range(NPAIR, 2 * NPAIR)) + list(range(NPAIR)):
                    t = per.tile([128, KT * 128], BF, name=f"wqkm{m}", tag=f"wqkm{m}")
                    nc.scalar.dma_start(
                        out=t.rearrange("p (k c) -> p k c", k=KT),
                        in_=wqk_ext[m].rearrange("(k p) c -> p k c", p=128),
                    )
                    wqkm[m] = t
                rotq_sb = [
                    per.tile([128, NL], BF, name=f"rotq{m}", tag=f"rotq{m}")
                    for m in range(NPAIR)
                ]

                def qk_tile(m, dest):
                    # m: M-tile index into [0, 18): 0-8 q slots, 9-17 k slots
                    pqk = psP.tile([128, NL], F32, name="qk", tag="qk", bufs=3)
                    for k in range(KT):
                        nc.tensor.matmul(
                            pqk[:],
                            lhsT=wqkm[m][:, k * 128:(k + 1) * 128],
                            rhs=xT_sb[k][:],
                            start=(k == 0),
                            stop=(k == KT - 1),
                        )
                    qkbf = wrk.tile([128, NL], BF, name="qkbf", tag="qkbf")
                    nc.vector.tensor_copy(qkbf[:], pqk[:])
                    psw = psP.tile([128, NL], F32, name="sw", tag="sw")
                    nc.tensor.matmul(psw[:], lhsT=perm_sb[:], rhs=qkbf[:])
                    t1 = wrk.tile([128, NL], BF, name="t1", tag="t1")
                    nc.vector.tensor_tensor(t1[:], qkbf[:], cos_sb[:], ALU.mult)
                    t2 = wrk.tile([128, NL], BF, name="t2", tag="t2")
                    nc.vector.tensor_tensor(t2[:], psw[:], sin_sb[:], ALU.mult)
                    nc.vector.tensor_tensor(dest[:], t1[:], t2[:], ALU.add)
                    if dbg == "rot":
                        dt_ = wrk.tile([128, NL], F32, name="dbgc", tag="dbgc")
                        nc.vector.tensor_copy(dt_[:], dest[:])
                        nc.sync.dma_start(out=dbg_ext[m * 128:(m + 1) * 128, :], in_=dt_[:])
                    if m >= NPAIR:
                        p = m - NPAIR
                        j = CHUNK_OF_PAIR[p]
                        r0 = (p - FIRST_PAIR[j]) * 128
                        nc.sync.dma_start(out=kb_loc[j][r0:r0 + 128, :], in_=dest[:])

                def ag_fire_k(j):
                    nc.gpsimd.collective_compute(
                        "AllGather", ALU.bypass,
                        ins=[kb_loc[j].opt()], outs=[kb_gat[j].opt()],
                        replica_groups=RG,
                    )

                def ag_fire_v(j):
                    nc.gpsimd.collective_compute(
                        "AllGather", ALU.bypass,
                        ins=[vb_loc[j].opt()], outs=[vb_gat[j].opt()],
                        replica_groups=RG,
                    )

                if dbg == "xT":
                    for k in range(KT):
                        dt_ = wrk.tile([128, NL], F32, name="dbgc", tag="dbgc")
                        nc.vector.tensor_copy(dt_[:], xT_sb[k][:])
                        nc.sync.dma_start(out=dbg_ext[k * 128:(k + 1) * 128, :], in_=dt_[:])

                # ---------- v projection -> vx (chunk-sliced bounces) ----------
                VN = [(0, 480, 8), (480, 960, 8), (960, 1020, 1)]  # (c0, c1, nheads)
                for mt in range(4):
                    vxt = wrk.tile([128, VX], BF, name="vx", tag="vx")
                    ones_ap = vxt.rearrange("p (h c) -> p h c", c=DH + 1)[:, :, 0:1]
                    nc.vector.memset(ones_ap, 1.0)
                    for (c0, c1, nh) in VN:
                        pv = psP.tile([128, 480], F32, name="pv", tag="qk", bufs=3)
                        for k in range(KT):
                            nc.tensor.matmul(
                                pv[:, 0:c1 - c0],
                                lhsT=xT_sb[k][:, mt * 128:(mt + 1) * 128],
                                rhs=wv_sb[k][:, c0:c1],
                                start=(k == 0),
                                stop=(k == KT - 1),
                            )
                        h0 = c0 // DH
                        src = pv[:, 0:c1 - c0].rearrange("p (h d) -> p h d", d=DH)
                        dst = vxt[:, h0 * 61:(h0 + nh) * 61].rearrange(
                            "p (h c) -> p h c", c=DH + 1
                        )[:, :, 1:DH + 1]
                        nc.vector.tensor_copy(dst, src)
                    for j in range(4):
                        fh, nh_ = FIRST_HEAD[j], NHEADS_OF[j]
                        nc.sync.dma_start(
                            out=vb_loc[j][mt * 128:(mt + 1) * 128, :],
                            in_=vxt[:, fh * 61:(fh + nh_) * 61],
                        )

                for m in (9,):
                    dest = wrk.tile([128, NL], BF, name="rotk", tag="rotk")
                    qk_tile(m, dest)
                ag_fire_k(0)
                ag_fire_v(0)
                qk_tile(0, rotq_sb[0])
                for m in (10, 11):
                    dest = wrk.tile([128, NL], BF, name="rotk", tag="rotk")
                    qk_tile(m, dest)
                ag_fire_v(1)
                ag_fire_k(1)
                for m in (12, 13, 14):
                    dest = wrk.tile([128, NL], BF, name="rotk", tag="rotk")
                    qk_tile(m, dest)
                ag_fire_v(2)
                ag_fire_k(2)
                for m in (15, 16, 17):
                    dest = wrk.tile([128, NL], BF, name="rotk", tag="rotk")
                    qk_tile(m, dest)
                ag_fire_v(3)
                ag_fire_k(3)

                # ---------- remaining q-slot projection + rope ----------
                for m in range(1, NPAIR):
                    qk_tile(m, rotq_sb[m])

            # load w_out late (not needed until the end)
            wout_sb = []
            for k in range(NPAIR):
                t = per.tile([128, DIM], BF, name=f"wout{k}", tag=f"wout{k}")
                nc.sync.dma_start(out=t[:], in_=wout_ext[k * 128:(k + 1) * 128, :])
                wout_sb.append(t)

            # ---------- attention ----------
            aoT = [
                per.tile([128, NL], BF, name=f"aoT{p}", tag=f"aoT{p}")
                for p in range(NPAIR)
            ]
            for p in range(NPAIR):
                nc.gpsimd.memset(aoT[p][:], 0.0)
            # bias row (inner index 1088 = slot 17 row 0 -> tile 8, partition 64)
            nc.vector.memset(aoT[NPAIR - 1][64:65, :], 1.0)

            # gathered V tiles resident in SBUF, grouped by AG chunk
            vxg = {}
            for j in range(4):
                vcols = NHEADS_OF[j] * 61
                for c in range(16):
                    t = per.tile([128, vcols], BF, name=f"vxg{j}_{c}", tag=f"vxg{j}_{c}")
                    nc.sync.dma_start(out=t[:], in_=vb_gat[j][c * 128:(c + 1) * 128, :])
                    vxg[(j, c)] = t

            with (
                tc.tile_pool(name="psD", bufs=2, space="PSUM") as psD,
                tc.tile_pool(name="psAV", bufs=4, space="PSUM") as psAV,
            ):
                def pair_setup(p):
                    j = CHUNK_OF_PAIR[p]
                    rows_j = len(PAIRS_OF[j]) * 128
                    pr0 = (p - FIRST_PAIR[j]) * 128
                    ktp = wrk.tile([128, 4 * NL], BF, name="ktp", tag="ktp", bufs=4)
                    for r in range(4):
                        nc.sync.dma_start(
                            out=ktp[:, r * NL:(r + 1) * NL],
                            in_=kb_gat[j][r * rows_j + pr0: r * rows_j + pr0 + 128, :],
                        )
                    avA = psAV.tile([128, NL], F32, name="avA", tag="av")
                    avB = (
                        psAV.tile([128, NL], F32, name="avB", tag="av")
                        if p < NPAIR - 1 else None
                    )
                    return ktp, avA, avB

                def pair_chunk(p, c, ktp, avA, avB):
                    two = avB is not None
                    j = CHUNK_OF_PAIR[p]
                    hA, hB = 2 * p, 2 * p + 1
                    lA = (hA - FIRST_HEAD[j]) * 61
                    lB = (hB - FIRST_HEAD[j]) * 61
                    dots = psD.tile([128, 1024], F32, name="dots", tag="dots")
                    nc.tensor.matmul(
                        dots[:, 0:NL],
                        lhsT=ktp[0:DH, c * 128:(c + 1) * 128],
                        rhs=rotq_sb[p][0:DH, :],
                    )
                    if two:
                        nc.tensor.matmul(
                            dots[:, NL:2 * NL],
                            lhsT=ktp[64:64 + DH, c * 128:(c + 1) * 128],
                            rhs=rotq_sb[p][64:64 + DH, :],
                        )
                    if dbg == "dots0" and p == 0:
                        dt_ = wrk.tile([128, 1024], F32, name="dbgd", tag="dbgd")
                        nc.vector.tensor_copy(dt_[:], dots[:])
                        nc.sync.dma_start(
                            out=dbg_ext[c * 128:(c + 1) * 128, :], in_=dt_[:]
                        )
                    et = expp.tile([128, 1024], BF, name="et", tag="expT", bufs=6)
                    width = 1024 if two else NL
                    nc.scalar.activation(
                        et[:, 0:width], dots[:, 0:width], AF.Exp, scale=SCALE
                    )
                    nc.tensor.matmul(
                        avA[0:61, :],
                        lhsT=vxg[(j, c)][:, lA:lA + 61],
                        rhs=et[:, 0:NL],
                        start=(c == 0),
                        stop=(c == 15),
                    )
                    if two:
                        nc.tensor.matmul(
                            avB[64:125, :],
                            lhsT=vxg[(j, c)][:, lB:lB + 61],
                            rhs=et[:, NL:2 * NL],
                            start=(c == 0),
                            stop=(c == 15),
                        )

                def pair_epilogue(p, avA, avB):
                    two = avB is not None
                    if dbg == "av0" and p == 0:
                        dt_ = wrk.tile([128, NL], F32, name="dbgd", tag="dbgd")
                        nc.vector.tensor_copy(dt_[:], avA[:])
                        nc.sync.dma_start(out=dbg_ext[0:128, :], in_=dt_[:])
                        dt2_ = wrk.tile([128, NL], F32, name="dbgd2", tag="dbgd2")
                        nc.vector.tensor_copy(dt2_[:], avB[:])
                        nc.sync.dma_start(out=dbg_ext[128:256, :], in_=dt2_[:])
                    rcA = wrk.tile([1, NL], F32, name="rc", tag="rc")
                    rcA_s = wrk.tile([1, NL], F32, name="rcs", tag="rcs")
                    nc.vector.tensor_copy(rcA_s[:], avA[0:1, :])
                    nc.vector.reciprocal_approx_fast(rcA[:], rcA_s[:])
                    bc = wrk.tile([128, NL], F32, name="bc", tag="bc")
                    nc.gpsimd.partition_broadcast(bc[0:61, :], rcA[:])
                    nc.vector.tensor_tensor(
                        aoT[p][0:61, :], avA[0:61, :], bc[0:61, :], ALU.mult
                    )
                    if two:
                        rcB = wrk.tile([1, NL], F32, name="rc", tag="rc")
                        rcB_s = wrk.tile([1, NL], F32, name="rcs", tag="rcs")
                        nc.vector.tensor_copy(rcB_s[:], avB[64:65, :])
                        nc.vector.reciprocal_approx_fast(rcB[:], rcB_s[:])
                        bc2 = wrk.tile([128, NL], F32, name="bc2", tag="bc2")
                        nc.gpsimd.partition_broadcast(bc2[0:61, :], rcB[:])
                        nc.vector.tensor_tensor(
                            aoT[p][64:125, :],
                            avB[64:125, :],
                            bc2[0:61, :],
                            ALU.mult,
                        )

                for pp in range(0, NPAIR, 1):
                    group = [p for p in (pp,)]
                    state = {p: pair_setup(p) for p in group}
                    for c in range(16):
                        for p in group:
                            ktp, avA, avB = state[p]
                            pair_chunk(p, c, ktp, avA, avB)
                    for p in group:
                        _, avA, avB = state[p]
                        pair_epilogue(p, avA, avB)

            if dbg == "aoT":
                for i in range(NPAIR):
                    dt2_ = wrk.tile([128, NL], F32, name="dbga", tag="dbga")
                    nc.vector.tensor_copy(dt2_[:], aoT[i][:])
                    nc.sync.dma_start(out=dbg_ext[i * 128:(i + 1) * 128, :], in_=dt2_[:])

            # ---------- output projection ----------
            with tc.tile_pool(name="psO", bufs=2, space="PSUM") as psO:
                for mt in range(4):
                    for (n0, n1) in ((0, 510), (510, 1020)):
                        po = psO.tile([128, 510], F32, name="po", tag="o")
                        for kt in range(NPAIR):
                            nc.tensor.matmul(
                                po[:],
                                lhsT=aoT[kt][:, mt * 128:(mt + 1) * 128],
                                rhs=wout_sb[kt][:, n0:n1],
                                start=(kt == 0),
                                stop=(kt == NPAIR - 1),
                            )
                        ot = wrk.tile([128, 510], F32, name="ot", tag="ot")
                        nc.vector.tensor_copy(ot[:], po[:])
                        nc.sync.dma_start(
                            out=out_ext[mt * 128:(mt + 1) * 128, n0:n1], in_=ot[:]
                        )

    nc.finalize()
    return nc


def _host_prep(x, coords, w_qkv, w_out, b_out):
    bf16 = ml_dtypes.bfloat16
    x = np.asarray(x, np.float32)
    coords = np.asarray(coords, np.float32)
    w_qkv = np.asarray(w_qkv, np.float32)
    w_out = np.asarray(w_out, np.float32)
    b_out = np.asarray(b_out, np.float32)

    # weights: q/k into 64-wide head slots, [1024, 2*1152]
    wqk = np.zeros((DIMP, 2 * MQK), np.float32)
    wq = w_qkv[:, 0:DIM].reshape(DIM, HEADS, DH)
    wk = w_qkv[:, DIM:2 * DIM].reshape(DIM, HEADS, DH)
    t = np.zeros((DIM, NSLOT, SLOT), np.float32)
    t[:, :HEADS, :DH] = wq
    wqk[:DIM, 0:MQK] = t.reshape(DIM, MQK)
    t[:] = 0.0
    t[:, :HEADS, :DH] = wk
    wqk[:DIM, MQK:2 * MQK] = t.reshape(DIM, MQK)
    # M-tile-major: [18, 1024, 128]
    wqk = np.ascontiguousarray(
        wqk.reshape(DIMP, 2 * NPAIR, 128).transpose(1, 0, 2)
    ).astype(bf16)

    wv = np.zeros((DIMP, DIM), np.float32)
    wv[:DIM, :] = w_qkv[:, 2 * DIM:3 * DIM]
    wv = wv.astype(bf16)

    wout = np.zeros((NSLOT, SLOT, DIM), np.float32)
    wout[:HEADS, 1:DH + 1, :] = w_out.reshape(HEADS, DH, DIM)
    wout[NSLOT - 1, 0, :] = b_out  # bias row at inner index 1088
    wout = wout.reshape(MQK, DIM).astype(bf16)

    # permutation matrix: out[m] = q[partner(m)] (rotate-half pair swap)
    perm = np.zeros((128, 128), np.float32)
    for m in range(128):
        a = m % SLOT
        if a < DH:
            pos = a % D3
            partner = (m // SLOT) * SLOT + (a // D3) * D3 + (
                pos + 10 if pos < 10 else pos - 10
            )
            perm[partner, m] = 1.0
    perm = perm.astype(bf16)

    # rotary tables per core: [128, 512] two identical 64-row head slots
    inv_freq = 1.0 / (10000.0 ** (np.arange(0, D3, 2, dtype=np.float32) / D3))  # [10]
    j = np.arange(SLOT)
    axis_of = np.clip(j // D3, 0, 2)
    jj = (j % D3) % 10
    sign = np.where((j % D3) < 10, -1.0, 1.0).astype(np.float32)
    valid = (j < DH).astype(np.float32)

    in_maps = []
    outs_meta = []
    for c in range(8):
        g, r = c // 4, c % 4
        rows = slice(r * NL, (r + 1) * NL)
        x_loc = np.ascontiguousarray(x[g, rows, :]).astype(bf16)
        t_axis = coords[g, rows, :]  # [NL, 3]
        f = (t_axis[:, axis_of] / MIN_FREQ) * inv_freq[jj][None, :]  # [NL, 64]
        cos_t = (np.cos(f) * valid[None, :]).T.astype(np.float32)  # [64, NL]
        sin_t = (np.sin(f) * (sign * valid)[None, :]).T.astype(np.float32)
        cos_full = np.concatenate([cos_t, cos_t], axis=0).astype(bf16)  # [128, NL]
        sin_full = np.concatenate([sin_t, sin_t], axis=0).astype(bf16)
        in_maps.append({
            "x": x_loc,
            "wqk": wqk,
            "wv": wv,
            "wout": wout,
            "cos_t": cos_full,
            "sin_t": sin_full,
            "perm": perm,
        })
        outs_meta.append((g, rows))
    return in_maps, outs_meta


def kernel(x, coords, w_qkv, w_out, b_out, _trace=False, _dbg=None):
    from concourse import bass_utils

    in_maps, outs_meta = _host_prep(x, coords, w_qkv, w_out, b_out)
    key = _dbg or "nc"
    if key not in _nc_cache:
        _nc_cache[key] = _build_nc(dbg=_dbg)
    nc = _nc_cache[key]
    last_err = None
    for _attempt in range(3):
        try:
            res = bass_utils.run_bass_kernel_spmd(
                nc, in_maps, core_ids=list(range(8)), trace=_trace
            )
            break
        except Exception as e:  # transient axon worker failures
            last_err = e
            import time as _time
            _time.sleep(2.0)
    else:
        raise last_err
    if _dbg is not None:
        kernel.dbg_results = [r["dbg"] for r in res.results]
    out = np.empty((B, N, DIM), np.float32)
    for c, (g, rows) in enumerate(outs_meta):
        out[g, rows, :] = res.results[c]["out"]
    if _trace:
        kernel.last_exec_time_ns = res.exec_time_ns
    return out

